# revision 1
# baseline (speedup 1.0000x reference)
"""Trainium2 Bass kernel for nn_CustomMatrixMultiplication.

Computes out[b, m] = sum_{n,p} m1[b, n, m] * m2[b, p, n]
              = sum_n m1[b, n, m] * s[b, n],   s[b, n] = sum_p m2[b, p, n]

Sharding: pure data parallel over batch B=64 across 8 NeuronCores
(8 batches per core). Each core:
  stage 1: s = ones.T @ m2[b]   (PE matmul, f32r, partition-dim reduction)
  relayout: s [1,1024] -> sT [128,8] via tiny SBUF->SBUF scatter DMA
            (row p=8i+r of the contraction lives on partition i, so both
             stages use the same mod-8 row grouping and no transpose is
             needed anywhere)
  stage 2: out = sum_g sT[:,g].T @ m1tile[g]  (PE matmul, f32r)
f32r (tf32-like, ~1e-4 rel) keeps the PE at 1 cycle/row for 512-wide
moving operands; accumulation is fp32 in PSUM.
"""

from contextlib import ExitStack

import numpy as np

import concourse.bacc as bacc
import concourse.mybir as mybir
import concourse.tile as tile
from concourse.bass_utils import run_bass_kernel_spmd

dt = mybir.dt

B, N, M, P = 64, 1024, 1024, 1024
NCORES = 8
BL = B // NCORES  # batches per core
H = 512           # matmul free-dim tile (fp32 moving-operand max)
R = 8             # row groups of 128 (1024 contraction rows / 128 partitions)
R2 = R // 2       # row groups per 2MB load half

_cache = {}


def _build():
    nc = bacc.Bacc(None, target_bir_lowering=False)
    m1_d = nc.dram_tensor("matrix1", [BL, N, M], dt.float32r, kind="ExternalInput")
    m2_d = nc.dram_tensor("matrix2", [BL, P, N], dt.float32r, kind="ExternalInput")
    out_d = nc.dram_tensor("out", [BL, M], dt.float32, kind="ExternalOutput")

    with tile.TileContext(nc) as tc, ExitStack() as ctx:
        m2p = ctx.enter_context(tc.tile_pool(name="m2p", bufs=4))
        m1h = ctx.enter_context(tc.tile_pool(name="m1h", bufs=4))
        m1q = ctx.enter_context(tc.tile_pool(name="m1q", bufs=4))
        small = ctx.enter_context(tc.tile_pool(name="small", bufs=2))
        stp = ctx.enter_context(tc.tile_pool(name="stp", bufs=3))
        const = ctx.enter_context(tc.tile_pool(name="const", bufs=1))
        psum = ctx.enter_context(tc.tile_pool(name="psum", bufs=3, space="PSUM"))

        ones_f32 = const.tile([128, 1], dt.float32)
        nc.vector.memset(ones_f32[:], 1.0)
        ones = const.tile([128, 1], dt.float32r)
        nc.vector.tensor_copy(ones[:], ones_f32[:])

        m1ts = [None] * BL
        sTs = [None] * BL

        def stage1(b):
            # load m2[b] in two 2MB halves: row 8i+r -> partition i,
            # free [r, n]; contiguous source (128 descriptors x 16KB each)
            m2_ap = m2_d[b].rearrange("(p r) n -> p r n", p=128)
            ps_s = psum.tile([1, N], dt.float32, tag="ps")
            for half in range(2):
                m2t = m2p.tile([128, R2, N], dt.float32r, tag="m2")
                nc.sync.dma_start(m2t[:], m2_ap[:, half * R2 : half * R2 + R2, :])
                # stage 1: s[n] = sum_r sum_i m2[8i+r, n]
                for h in range(N // H):
                    for r in range(half * R2, half * R2 + R2):
                        nc.tensor.matmul(
                            ps_s[0:1, H * h : H * (h + 1)],
                            ones[:],
                            m2t[:, r - half * R2, H * h : H * (h + 1)],
                            start=(r == 0),
                            stop=(r == R - 1),
                        )
            s_b = small.tile([1, N], dt.float32r, tag="s")
            nc.vector.tensor_copy(s_b[:], ps_s[:])  # rounds to f32r
            # relayout: sT[i, g] = s[8i + g]
            sT = stp.tile([128, R], dt.float32r, tag="sT")
            nc.scalar.dma_start(sT[:], s_b[:])
            sTs[b] = sT

        def m1load(b, nparts):
            # load m1[b] with the same mod-8 row grouping
            m1_ap = m1_d[b].rearrange("(p r) m -> p r m", p=128)
            rr = R // nparts
            pool, tag = (m1h, "m1h") if nparts == 2 else (m1q, "m1q")
            parts = []
            for q in range(nparts):
                m1t = pool.tile([128, rr, M], dt.float32r, tag=tag)
                nc.sync.dma_start(m1t[:], m1_ap[:, q * rr : (q + 1) * rr, :])
                parts.append(m1t)
            m1ts[b] = parts

        def stage2(b):
            # stage 2: out[m] = sum_g sum_i m1[8i+g, m] * s[8i+g]
            # per-h-slice PSUM copies start as soon as that h's
            # accumulation chain stops, then one 4KB out DMA per batch
            sT, parts = sTs[b], m1ts[b]
            rr = R // len(parts)
            ps_o = psum.tile([1, M], dt.float32, tag="ps")
            o_b = small.tile([1, M], dt.float32, tag="o")
            for q, m1t in enumerate(parts):
                for h in range(M // H):
                    for g in range(q * rr, (q + 1) * rr):
                        nc.tensor.matmul(
                            ps_o[0:1, H * h : H * (h + 1)],
                            sT[:, g : g + 1],
                            m1t[:, g - q * rr, H * h : H * (h + 1)],
                            start=(g == 0),
                            stop=(g == R - 1),
                        )
                    if q == len(parts) - 1:
                        nc.vector.tensor_copy(
                            o_b[0:1, H * h : H * (h + 1)],
                            ps_o[0:1, H * h : H * (h + 1)],
                        )
            nc.scalar.dma_start(out_d[b : b + 1, :], o_b[:])

        # Schedule: m2[7] loads+reduces early so the kernel tail is only
        # m1[7]'s last 1MB quarter -> 4 matmuls -> copy -> 4KB out DMA.
        stage1(0)
        stage1(BL - 1)
        for b in range(BL - 1):
            m1load(b, nparts=2)
            if b < BL - 2:
                stage1(b + 1)
            stage2(b)
        m1load(BL - 1, nparts=4)
        stage2(BL - 1)

    nc.finalize()
    return nc


def _get_nc():
    if "nc" not in _cache:
        _cache["nc"] = _build()
    return _cache["nc"]


def kernel(matrix1, matrix2, _run_kwargs=None):
    m1 = np.ascontiguousarray(np.asarray(matrix1, dtype=np.float32))
    m2 = np.ascontiguousarray(np.asarray(matrix2, dtype=np.float32))
    assert m1.shape == (B, N, M) and m2.shape == (B, P, N)

    nc = _get_nc()
    in_maps = [
        {
            "matrix1": m1[i * BL : (i + 1) * BL],
            "matrix2": m2[i * BL : (i + 1) * BL],
        }
        for i in range(NCORES)
    ]
    res = run_bass_kernel_spmd(
        nc, in_maps, core_ids=list(range(NCORES)), **(_run_kwargs or {})
    )
    out = np.concatenate([res.results[i]["out"] for i in range(NCORES)], axis=0)
    if _run_kwargs:
        _cache["last_results"] = res
    return out

